# revision 25
# baseline (speedup 1.0000x reference)
"""DeepSeekV3-style MoE layer (1 MoE block) on 8 Trainium2 NeuronCores.

v4: sparse expert-parallel. The router (0.5% of the FLOPs) runs on the host
during input sharding; each core receives, for each of its 4 local experts,
only the tokens that actually routed to it (capacity 1152 = mean 1024 + 4.6
sigma, padded slots carry combine-weight 0), pre-transposed to feature-major
bf16. The device computes just the expert FFNs -- a 3.3x MAC reduction vs
the dense-all-experts formulation -- plus the full shared expert for a
512-token data-parallel slice, so no collectives are needed at all. The
combine weight is applied on-device to each expert's down-projection output;
the down-bias term w*bd and the shared bias bd_s are added on the host
(y += w_full @ bd + bd_s), and the host sums the weighted per-slot outputs.

v4 changes vs v3 (518us):
  - capacity split 3x384 instead of 512/512/128: every stationary weight
    tile is loaded once per expert and reused for 3 moving chunks, so
    LDWEIGHTS amortizes and the PE issue stream is dense (HAM stays warm)
  - no more K=1 bias matmuls (~90 of them): bd handled on host
  - big weights land in per-h / per-i tiles so the first matmuls only wait
    on a 128-256KB DMA instead of a 1-2MB one (kills the startup stall)
v5 changes vs v4 (337us):
  - the Scalar engine issues NO DMAs: its 60 weight loads were flow-
    controlled by transfer completions and held the first silu back to
    t=101us, stalling the whole PSUM pipeline behind the ACT engine.
    Weights now load on sync/gpsimd only.
  - down-projection PSUM drains split between ACT (Copy with per-partition
    scale = combine weight) and DVE so neither engine gates the down phase
  - shared-expert PSUM tags rotate so iteration i+1 never waits on the
    silu of iteration i
v6 changes vs v5 (294us):
  - xs/wgus loads h-interleaved and xg staged as one 288KB tile per h,
    so the first matmul starts ~10us earlier and expert-0 never waits
  - the last gate/up iteration's chunk-1/2 drains are deferred into the
    down phase, removing the ACT/DVE backlog stall at each down start
v7 changes vs v6 (289us):
  - shared expert moved to the END: its compute is dense only once all
    DMAs have long landed, and the slow first-40us DMA window is hidden
    under routed-expert matmuls instead of stalling the shared phase
  - sync queue strictly in consumption order (xg-e0, wu0/wd0, xg-e1,
    wu1/wd1, xs/wgus, wu2/wd2, wu3/wd3)
  - outp bufs 2->4: the 256KB output stores have ~2.6us SWDGE latency vs
    a 1.75us per-tile production rate in the down phase
"""

import sys

sys.path.insert(0, "/opt/trn_rl_repo")

import numpy as np

import concourse.bacc as bacc
import concourse.bass as bass
import concourse.mybir as mybir
import concourse.tile as tile

F32 = mybir.dt.float32
BF16 = mybir.dt.bfloat16
F8 = mybir.dt.float8e4
AF = mybir.ActivationFunctionType
ALU = mybir.AluOpType

H, I, E, TOPK = 1024, 512, 32, 8
B, S = 4, 1024
T = B * S
NCORES = 8
E_LOC = E // NCORES          # 4 routed experts per core
P = 128
NH = H // P                  # 8 hidden k-tiles
NI = I // P                  # 4 intermediate tiles
CAP = 1152                   # token capacity per expert (mean 1024 + 4.6 sigma)
NT_E = CAP // P              # 9 slot-tiles per expert
NSLOT = E_LOC * CAP          # 4608 slots per core
NTILE = E_LOC * NT_E         # 36 slot-tiles per core
TS = T // NCORES             # 512 shared-expert tokens per core
NC = 3                       # chunks per expert
CN = CAP // NC               # 384 slots per chunk
SWD = 64.0                   # host fp8 scale on Wd/Wd_s
SHS = 8.0                    # host scale on Wu/bu (makes hge=8h)


def build_nc():
    nc = bacc.Bacc(None, target_bir_lowering=False, num_devices=NCORES)

    xg_d = nc.declare_dram_parameter("xg", [NH, P, NSLOT], BF16, isOutput=False)
    xs_d = nc.declare_dram_parameter("xs", [NH, P, TS], BF16, isOutput=False)
    wg_d = nc.declare_dram_parameter("wg", [E_LOC, P, NH, I], BF16, isOutput=False)
    wu_d = nc.declare_dram_parameter("wu", [E_LOC, P, NH, I], BF16, isOutput=False)
    wd_d = nc.declare_dram_parameter("wd", [E_LOC, P, NI, H], F8, isOutput=False)
    bg_d = nc.declare_dram_parameter("bg", [P, E_LOC, NI], F32, isOutput=False)
    bu_d = nc.declare_dram_parameter("bu", [P, E_LOC, NI], F32, isOutput=False)
    wgus_d = nc.declare_dram_parameter("wgus", [P, NH, 2 * I], BF16, isOutput=False)
    wds_d = nc.declare_dram_parameter("wds", [P, NI, H], BF16, isOutput=False)
    bgus_d = nc.declare_dram_parameter("bgus", [P, 2 * NI], F32, isOutput=False)
    # combine weight per slot, tile-major: wcol[p, jt] = w of slot jt*128+p
    wcol_d = nc.declare_dram_parameter("wcol", [P, NTILE], F32, isOutput=False)
    yg_d = nc.declare_dram_parameter("yg", [NSLOT, H], BF16, isOutput=True)
    ys_d = nc.declare_dram_parameter("ys", [TS, H], BF16, isOutput=True)

    with tile.TileContext(nc) as tc:
        with (
            tc.tile_pool(name="wres", bufs=1) as wres,
            tc.tile_pool(name="xsb", bufs=1) as xsb,
            tc.tile_pool(name="xtb", bufs=2) as xtb,
            tc.tile_pool(name="hgep", bufs=2) as hgep,
            tc.tile_pool(name="hgsp", bufs=1) as hgsp,
            tc.tile_pool(name="actp", bufs=2) as actp,
            tc.tile_pool(name="outp", bufs=4) as outp,
            tc.tile_pool(name="ps_g", bufs=1, space="PSUM") as ps_g,
            tc.tile_pool(name="ps_u", bufs=1, space="PSUM") as ps_u,
            tc.tile_pool(name="ps_d", bufs=2, space="PSUM") as ps_d,
        ):
            # ---------- small constants (gpsimd = SWDGE ring, first) ---------
            bg_sb = wres.tile([P, E_LOC, NI], F32, tag="bg")
            nc.gpsimd.dma_start(bg_sb[:], bg_d[:])
            bu_sb = wres.tile([P, E_LOC, NI], F32, tag="bu")
            nc.gpsimd.dma_start(bu_sb[:], bu_d[:])
            wcol_sb = wres.tile([P, NTILE], F32, tag="wcol")
            nc.gpsimd.dma_start(wcol_sb[:], wcol_d[:])
            bgus_sb = wres.tile([P, 2 * NI], F32, tag="bgus")
            nc.gpsimd.dma_start(bgus_sb[:], bgus_d[:])
            # NOTE: no dma_start may ever be issued from the Scalar engine --
            # the silu activations queue behind them in its FIFO and DMA
            # issues are flow-controlled by transfer completions (measured:
            # first silu delayed to t=101us by 60 queued weight loads).
            # routed expert weights: wg1-3 on gpsimd (loads while sync
            # stages x); wg0 goes on sync interleaved with xg-e0 below so the
            # very first gate matmuls never wait on the slower SWDGE ring
            wg_bf = {}
            wu_bf = {}
            wd_bf = {}
            for e in range(1, E_LOC):
                for h in range(NH):
                    t = wres.tile([P, I], BF16, tag=f"wg{e}_{h}", name="wg_h")
                    nc.gpsimd.dma_start(t[:], wg_d[e][:, h, :])
                    wg_bf[(e, h)] = t
            wds_sb = []
            for i in range(NI):
                t = wres.tile([P, H], BF16, tag=f"wds{i}", name="wds_i")
                nc.gpsimd.dma_start(t[:], wds_d[:, i, :])
                wds_sb.append(t)

            # ---------- gathered-x staging: one whole-capacity tile per h ----
            # (8 DMAs x 288KB per expert: big transfers, 2.3KB lines)
            def stage_expert(e):
                ts = {}
                for h in range(NH):
                    xt = xtb.tile([P, CAP], BF16, tag=f"xg{h}", name=f"xg{h}")
                    nc.sync.dma_start(xt[:], xg_d[h][:, e * CAP:(e + 1) * CAP])
                    ts[h] = xt
                return ts

            def load_wuwd(e):
                for h in range(NH):
                    t = wres.tile([P, I], BF16, tag=f"wu{e}_{h}", name="wu_h")
                    nc.sync.dma_start(t[:], wu_d[e][:, h, :])
                    wu_bf[(e, h)] = t
                t = wres.tile([P, NI, H], F8, tag=f"wd{e}", name="wd_e")
                nc.sync.dma_start(t[:], wd_d[e])
                wd_bf[e] = t

            # PE warmup: ~4.3us of dummy matmuls while the first DMAs land,
            # so HAM un-throttles to K=8/8 before the first real matmul
            warm = wres.tile([P, 512], BF16, tag="warm")
            nc.vector.memset(warm[:], 0.0)
            for _ in range(10):
                pw = ps_d.tile([P, 512], F32, tag="d", name="pwarm")
                nc.tensor.matmul(pw[:], warm[:, 0:P], warm[:],
                                 start=True, stop=True)

            # sync queue order = consumption order: e0 x + weights, e1 x +
            # weights, then the shared-expert tensors (shared runs LAST so the
            # slow first-100us DMA window is hidden under routed compute),
            # then the remaining expert weights
            staged = {}
            ts0 = {}
            for h in range(NH):
                t = wres.tile([P, I], BF16, tag=f"wg0_{h}", name="wg_h")
                nc.sync.dma_start(t[:], wg_d[0][:, h, :])
                wg_bf[(0, h)] = t
                xt = xtb.tile([P, CAP], BF16, tag=f"xg{h}", name=f"xg{h}")
                nc.sync.dma_start(xt[:], xg_d[h][:, 0:CAP])
                ts0[h] = xt
            staged[0] = ts0
            load_wuwd(0)
            staged[1] = stage_expert(1)
            load_wuwd(1)
            xs_sb = []
            wgus_sb = []
            for h in range(NH):
                t = xsb.tile([P, TS], BF16, tag=f"xs{h}")
                nc.sync.dma_start(t[:], xs_d[h])
                xs_sb.append(t)
                t = wres.tile([P, 2 * I], BF16, tag=f"wgus{h}", name="wgus_h")
                nc.sync.dma_start(t[:], wgus_d[:, h, :])
                wgus_sb.append(t)
            load_wuwd(2)
            load_wuwd(3)

            # ---------- routed experts over gathered slots -------------------
            # gate/up: one stationary load serves the 3 moving chunks; down:
            # one hge stationary serves both output halves' weight slices
            for e in range(E_LOC):
                if e + 2 < E_LOC:
                    staged[e + 2] = stage_expert(e + 2)
                xt = staged.pop(e)
                hge = {}
                deferred = {}

                def drain_gu(e, i, c, pg, pu):
                    ga = actp.tile([P, CN], F32, tag=f"gact{c}", name="ga")
                    nc.scalar.activation(ga[:], pg[:, 0:CN], AF.Silu,
                                         bias=bg_sb[:, e, i:i + 1])
                    # fp8 pair tile [P, 2, CN]: i-pair member in dim1, feeds
                    # the DoubleRow down matmul (K=256 per instruction)
                    k, m = divmod(i, 2)
                    if m == 0:
                        hge[(k, c)] = hgep.tile([P, 2, CN], F8,
                                                tag=f"hge{k}_{c}", name="ht")
                    nc.vector.scalar_tensor_tensor(
                        hge[(k, c)][:, m, :], pu[:, 0:CN], bu_sb[:, e, i:i + 1],
                        ga[:], ALU.add, ALU.mult)

                for i in range(NI):
                    pgs = [ps_g.tile([P, 512], F32, tag=f"g{c}", name="pg")
                           for c in range(NC)]
                    for h in range(NH):
                        for c in range(NC):
                            nc.tensor.matmul(pgs[c][:, 0:CN],
                                             wg_bf[(e, h)][:, i * P:(i + 1) * P],
                                             xt[h][:, c * CN:(c + 1) * CN],
                                             start=(h == 0),
                                             stop=(h == NH - 1))
                    pus = [ps_u.tile([P, 512], F32, tag=f"u{c}", name="pu")
                           for c in range(NC)]
                    for h in range(NH):
                        for c in range(NC):
                            nc.tensor.matmul(pus[c][:, 0:CN],
                                             wu_bf[(e, h)][:, i * P:(i + 1) * P],
                                             xt[h][:, c * CN:(c + 1) * CN],
                                             start=(h == 0),
                                             stop=(h == NH - 1))
                    for c in range(NC):
                        if i == NI - 1 and c > 0:
                            # defer the last iteration's chunk-1/2 drains into
                            # the down phase: the ACT/DVE backlog at down
                            # start otherwise stalls the PSUM rotation
                            deferred[c] = (pgs[c], pus[c])
                        else:
                            drain_gu(e, i, c, pgs[c], pus[c])
                for j in range(NT_E):
                    c, jc = divmod(j, CN // P)
                    if jc == 0 and c in deferred:
                        pg, pu = deferred.pop(c)
                        drain_gu(e, NI - 1, c, pg, pu)
                    jt = e * NT_E + j
                    out_sb = outp.tile([P, H], BF16, tag="out", name="yg_out")
                    for half in range(2):
                        h0 = half * (H // 2)
                        pd = ps_d.tile([P, 512], F32, tag="d", name="pd")
                        for k in range(2):
                            nc.tensor.matmul(
                                pd[:],
                                hge[(k, c)][:, :, jc * P:(jc + 1) * P],
                                wd_bf[e][:, 2 * k:2 * k + 2, h0:h0 + H // 2],
                                start=(k == 0), stop=(k == 1),
                                perf_mode=mybir.MatmulPerfMode.DoubleRow)
                        # combine-weight scale while draining PSUM; halves
                        # split between ACT (Copy w/ scale) and DVE so neither
                        # engine gates the down phase
                        if half == 0:
                            nc.scalar.activation(out_sb[:, h0:h0 + H // 2],
                                                 pd[:], AF.Copy,
                                                 scale=wcol_sb[:, jt:jt + 1])
                        else:
                            nc.vector.tensor_tensor(
                                out_sb[:, h0:h0 + H // 2], pd[:],
                                wcol_sb[:, jt:jt + 1].broadcast_to([P, H // 2]),
                                ALU.mult)
                    s0 = e * CAP + j * P
                    nc.gpsimd.dma_start(yg_d[s0:s0 + P, :], out_sb[:])

            # ---------- shared expert (tokens TS*core .. TS*(core+1)) --------
            TC = 512
            hs = []
            for i in range(NI):
                psg = ps_g.tile([P, TC], F32, tag=f"g{i % 3}", name="psg")
                for h in range(NH):
                    nc.tensor.matmul(psg[:], wgus_sb[h][:, i * P:(i + 1) * P],
                                     xs_sb[h][:], start=(h == 0),
                                     stop=(h == NH - 1))
                psu = ps_u.tile([P, TC], F32, tag=f"u{i % 3}", name="psu")
                for h in range(NH):
                    nc.tensor.matmul(psu[:], wgus_sb[h][:, I + i * P:I + (i + 1) * P],
                                     xs_sb[h][:], start=(h == 0),
                                     stop=(h == NH - 1))
                gs = actp.tile([P, TC], F32, tag="gact", name="gs")
                nc.scalar.activation(gs[:], psg[:], AF.Silu,
                                     bias=bgus_sb[:, i:i + 1])
                hsi = hgsp.tile([P, TC], BF16, tag=f"hs{i}")
                nc.vector.scalar_tensor_tensor(hsi[:], psu[:],
                                               bgus_sb[:, NI + i:NI + i + 1],
                                               gs[:], ALU.add, ALU.mult)
                hs.append(hsi)
            for j in range(TS // P):
                out_sb = outp.tile([P, H], BF16, tag="out", name="ys_out")
                for half in range(2):
                    h0 = half * (H // 2)
                    pd = ps_d.tile([P, 512], F32, tag="d", name="pds")
                    for i in range(NI):
                        nc.tensor.matmul(pd[:], hs[i][:, j * P:(j + 1) * P],
                                         wds_sb[i][:, h0:h0 + H // 2],
                                         start=(i == 0), stop=(i == NI - 1))
                    # split the PSUM drain between ACT and DVE
                    if half == 0:
                        nc.scalar.activation(out_sb[:, h0:h0 + H // 2], pd[:],
                                             AF.Copy)
                    else:
                        nc.vector.tensor_copy(out_sb[:, h0:h0 + H // 2], pd[:])
                nc.sync.dma_start(ys_d[j * P:(j + 1) * P, :], out_sb[:])

    nc.finalize()
    return nc


def _route(inputs):
    """Host-side router: top-8 selection, per-expert token lists, slot map."""
    x = np.ascontiguousarray(
        np.asarray(inputs["hidden_states"], np.float32)).reshape(T, H)
    Wr = np.asarray(inputs["Wr"], np.float32)
    br = np.asarray(inputs["br"], np.float32)
    logits = x @ Wr + br
    aff = 1.0 / (1.0 + np.exp(-logits))
    idx = np.argsort(-aff, axis=1, kind="stable")[:, :TOPK]        # [T, K]
    topv = np.take_along_axis(aff, idx, axis=1)
    topw = (topv / (topv.sum(1, keepdims=True) + 1e-9)).astype(np.float32)
    w_full = np.zeros((T, E), np.float32)
    np.put_along_axis(w_full, idx, topw, axis=1)

    tok_ids = np.full((E, CAP), -1, np.int64)   # token id per slot (-1 = pad)
    w_slot = np.zeros((E, CAP), np.float32)     # combine weight per slot
    # global slot index for each (token, expert) pair; -1 if not routed/dropped
    pos = np.full((T, E), -1, np.int64)
    for e in range(E):
        tl = np.nonzero(w_full[:, e] > 0)[0]
        if len(tl) > CAP:   # overflow: drop the smallest-weight tokens
            keep = np.argsort(-w_full[tl, e], kind="stable")[:CAP]
            tl = np.sort(tl[keep])
        c = e // E_LOC
        el = e % E_LOC
        base = c * NSLOT + el * CAP
        tok_ids[e, :len(tl)] = tl
        w_slot[e, :len(tl)] = w_full[tl, e]
        pos[tl, e] = base + np.arange(len(tl))
    slot_of = np.take_along_axis(pos, idx, axis=1)                 # [T, K]
    if (slot_of < 0).any():
        # dropped pairs: point at any zero-weight (padded) slot of the owning
        # core -- guaranteed to exist (sum of local loads <= T < NSLOT) and its
        # device output is exactly 0 (combine weight 0)
        flat_w = w_slot.reshape(NCORES, NSLOT)
        own_core = idx // E_LOC
        for c in range(NCORES):
            z = int(np.nonzero(flat_w[c] == 0)[0][0]) + c * NSLOT
            slot_of[(slot_of < 0) & (own_core == c)] = z
    return x, w_full, tok_ids, w_slot, slot_of


def prep(inputs):
    """Host routing + sharding: returns (per-core input maps, aux for assembly)."""
    import ml_dtypes
    bf = ml_dtypes.bfloat16

    x, w_full, tok_ids, w_slot, slot_of = _route(inputs)
    Wg = np.asarray(inputs["Wg"], np.float32)
    bg = np.asarray(inputs["bg"], np.float32)
    Wu = np.asarray(inputs["Wu"], np.float32)
    bu = np.asarray(inputs["bu"], np.float32)
    Wd = np.asarray(inputs["Wd"], np.float32)
    bd = np.asarray(inputs["bd"], np.float32)
    Wg_s = np.asarray(inputs["Wg_s"], np.float32)
    bg_s = np.asarray(inputs["bg_s"], np.float32)
    Wu_s = np.asarray(inputs["Wu_s"], np.float32)
    bu_s = np.asarray(inputs["bu_s"], np.float32)
    Wd_s = np.asarray(inputs["Wd_s"], np.float32)
    bd_s = np.asarray(inputs["bd_s"], np.float32)

    f8 = ml_dtypes.float8_e4m3

    xT = np.ascontiguousarray(x.T.astype(bf))                      # [H, T]
    # Wu/bu are pre-scaled by SHS so hge = SHS*h fits fp8 e4m3 well; Wd is
    # quantized to fp8 with scale SWD. Both scales are folded into wcol on
    # the host (and divided out of ys after the run).
    wgus = np.concatenate([Wg_s, Wu_s], axis=1)                    # [H, 2I]
    wgus_c = np.ascontiguousarray(
        wgus.reshape(NH, P, 2 * I).transpose(1, 0, 2).astype(bf))
    wds_c = np.ascontiguousarray(
        Wd_s.reshape(NI, P, H).transpose(1, 0, 2).astype(bf))
    bgus_c = np.ascontiguousarray(
        np.stack([bg_s.reshape(NI, P), bu_s.reshape(NI, P)], 0)
        .reshape(2 * NI, P).T)
    # host-side bias term: sum_e w[t,e]*bd[e] plus the shared expert's bd_s
    bias_host = w_full @ bd + bd_s                                 # [T, H]

    in_maps = []
    for c in range(NCORES):
        loc = list(range(c * E_LOC, (c + 1) * E_LOC))
        cols = tok_ids[loc].reshape(-1).clip(0)                    # [NSLOT]
        xg = xT[:, cols].reshape(NH, P, NSLOT)
        wcol = np.ascontiguousarray(
            w_slot[loc].reshape(NTILE, P).T / (SWD * SHS))         # [P,NTILE]
        in_maps.append({
            "xg": np.ascontiguousarray(xg),
            "xs": np.ascontiguousarray(
                xT[:, c * TS:(c + 1) * TS].reshape(NH, P, TS)),
            "wg": np.ascontiguousarray(
                Wg[loc].reshape(E_LOC, NH, P, I).transpose(0, 2, 1, 3).astype(bf)),
            "wu": np.ascontiguousarray(
                (SHS * Wu[loc]).reshape(E_LOC, NH, P, I).transpose(0, 2, 1, 3).astype(bf)),
            "wd": np.ascontiguousarray(
                (SWD * Wd[loc]).reshape(E_LOC, NI, P, H).transpose(0, 2, 1, 3).astype(f8)),
            "bg": np.ascontiguousarray(bg[loc].reshape(E_LOC, NI, P).transpose(2, 0, 1)),
            "bu": np.ascontiguousarray(
                SHS * bu[loc].reshape(E_LOC, NI, P).transpose(2, 0, 1)),
            "wgus": wgus_c,
            "wds": wds_c,
            "bgus": bgus_c,
            "wcol": wcol,
        })
    return in_maps, (slot_of, bias_host)


def prep_inputs(inputs):
    return prep(inputs)[0]


def assemble_output(results, aux):
    """shared slices + weighted routed contributions + host-side bias term."""
    slot_of, bias_host = aux
    y = np.empty((T, H), np.float32)
    for c in range(NCORES):
        y[c * TS:(c + 1) * TS] = results[c]["ys"].astype(np.float32)
    down = np.concatenate([results[c]["yg"] for c in range(NCORES)], axis=0)
    y += down[slot_of].astype(np.float32).sum(axis=1)
    y += bias_host
    return y


_CACHE = {}


def get_runner():
    """Build + jit once; returns run(in_maps) -> list of per-core output dicts."""
    if "run" in _CACHE:
        return _CACHE["run"]
    import jax
    from jax.sharding import Mesh, PartitionSpec
    from jax.experimental.shard_map import shard_map
    from concourse import bass2jax

    nc = build_nc()
    bass2jax.install_neuronx_cc_hook()

    in_names = []
    out_names = []
    out_avals = []
    partition_name = nc.partition_id_tensor.name if nc.partition_id_tensor else None
    for alloc in nc.m.functions[0].allocations:
        if not isinstance(alloc, mybir.MemoryLocationSet):
            continue
        name = alloc.memorylocations[0].name
        if alloc.kind == "ExternalInput":
            if name != partition_name:
                in_names.append(name)
        elif alloc.kind == "ExternalOutput":
            out_names.append(name)
            out_avals.append(
                jax.core.ShapedArray(tuple(alloc.tensor_shape),
                                     mybir.dt.np(alloc.dtype)))
    n_params = len(in_names)
    n_outs = len(out_names)
    all_names = in_names + out_names + ([partition_name] if partition_name else [])
    donate = tuple(range(n_params, n_params + n_outs))

    def _body(*args):
        operands = list(args)
        if partition_name is not None:
            operands.append(bass2jax.partition_id_tensor())
        return tuple(bass2jax._bass_exec_p.bind(
            *operands,
            out_avals=tuple(out_avals),
            in_names=tuple(all_names),
            out_names=tuple(out_names),
            lowering_input_output_aliases=(),
            sim_require_finite=True,
            sim_require_nnan=True,
            nc=nc,
        ))

    devices = jax.devices()[:NCORES]
    mesh = Mesh(np.asarray(devices), ("core",))
    in_specs = (PartitionSpec("core"),) * (n_params + n_outs)
    out_specs = (PartitionSpec("core"),) * n_outs
    sharded = jax.jit(
        shard_map(_body, mesh=mesh, in_specs=in_specs, out_specs=out_specs,
                  check_rep=False),
        donate_argnums=donate, keep_unused=True)

    def run(in_maps, dev_inputs=None):
        if dev_inputs is None:
            dev_inputs = [
                np.concatenate([np.asarray(in_maps[c][n]) for c in range(NCORES)],
                               axis=0)
                for n in in_names
            ]
        zeros = [np.zeros((NCORES * a.shape[0], *a.shape[1:]), a.dtype)
                 for a in out_avals]
        outs = sharded(*dev_inputs, *zeros)
        return [
            {name: np.asarray(outs[i]).reshape(NCORES, *out_avals[i].shape)[c]
             for i, name in enumerate(out_names)}
            for c in range(NCORES)
        ]

    _CACHE["run"] = run
    _CACHE["meta"] = (in_names, out_names, out_avals, sharded, mesh)
    return run


def kernel(**inputs) -> np.ndarray:
    run = get_runner()
    in_maps, aux = prep(inputs)
    results = run(in_maps)
    return assemble_output(results, aux).reshape(B, S, H).astype(np.float32)


# revision 28
# speedup vs baseline: 1.0308x; 1.0308x over previous
"""DeepSeekV3-style MoE layer (1 MoE block) on 8 Trainium2 NeuronCores.

Sparse expert-parallel. The router (0.5% of the FLOPs) runs on the host
during input sharding; each core receives, for each of its 4 local experts,
only the tokens that actually routed to it (capacity 1152 = mean 1024 + 4.6
sigma = 3 chunks of 384; padded slots carry combine-weight 0), pre-transposed
to feature-major bf16. The device computes just the expert FFNs -- a 3.3x MAC
reduction vs the dense-all-experts formulation -- plus the full shared expert
for a 512-token data-parallel slice, so no collectives are needed at all.
The combine weight is applied on-device to each expert's down-projection
output; the down-bias term w*bd and the shared bias bd_s are added on the
host (y += w_full @ bd + bd_s), and the host sums the weighted per-slot
outputs (slot_of map). Dropped-on-overflow pairs (never for the reference
input) fall back to a zero-weight slot, i.e. contribute 0.

Device-side structure (why it is fast -- 233us vs the 1135us dense baseline):
  - every stationary weight tile is loaded once and feeds 3 moving chunks
    (3x384 split), so LDWEIGHTS amortizes, the PE issue stream is dense and
    the HAM clock gate stays at K=8/8; a 10-matmul warmup burst un-throttles
    the PE while the first DMAs land
  - the routed down-projection runs in fp8e4m3 with perf_mode=DoubleRow
    (K=256 per instruction, ~2x): Wu/bu are host-scaled by 8 so hge=8h fits
    e4m3, Wd is host-quantized with scale 64, and both scales are folded
    into the combine weight / host bias. The shared expert stays bf16 (its
    output is too large a fraction of y to survive fp8). rel_fro ~1.3e-2
    vs the 2e-2 gate.
  - the Scalar engine issues NO DMAs (its queue must stay free for silu:
    DMA issues are flow-controlled by transfer completions and once delayed
    the first silu to t=101us); all loads go on sync/gpsimd in consumption
    order, outputs on gpsimd, with per-h weight tiles so first matmuls wait
    on 128-256KB transfers only
  - PSUM: 3 gate + 3 up banks (one per chunk) + 2 down banks; the down
    drain is split between ACT (Copy with per-partition scale = combine
    weight) and DVE, and the last gate/up iteration's chunk-1/2 drains are
    deferred into the down phase so neither engine backlog stalls the PE
  - the shared expert runs LAST, hiding the slow first-40us DMA window
    under routed-expert matmuls
"""

import sys

sys.path.insert(0, "/opt/trn_rl_repo")

import numpy as np

import concourse.bacc as bacc
import concourse.bass as bass
import concourse.mybir as mybir
import concourse.tile as tile

F32 = mybir.dt.float32
BF16 = mybir.dt.bfloat16
F8 = mybir.dt.float8e4
AF = mybir.ActivationFunctionType
ALU = mybir.AluOpType

H, I, E, TOPK = 1024, 512, 32, 8
B, S = 4, 1024
T = B * S
NCORES = 8
E_LOC = E // NCORES          # 4 routed experts per core
P = 128
NH = H // P                  # 8 hidden k-tiles
NI = I // P                  # 4 intermediate tiles
CAP = 1152                   # token capacity per expert (mean 1024 + 4.6 sigma)
NT_E = CAP // P              # 9 slot-tiles per expert
NSLOT = E_LOC * CAP          # 4608 slots per core
NTILE = E_LOC * NT_E         # 36 slot-tiles per core
TS = T // NCORES             # 512 shared-expert tokens per core
NC = 3                       # chunks per expert
CN = CAP // NC               # 384 slots per chunk
SWD = 64.0                   # host fp8 scale on Wd/Wd_s
SHS = 8.0                    # host scale on Wu/bu (makes hge=8h)


def build_nc():
    nc = bacc.Bacc(None, target_bir_lowering=False, num_devices=NCORES)

    xg_d = nc.declare_dram_parameter("xg", [NH, P, NSLOT], BF16, isOutput=False)
    xs_d = nc.declare_dram_parameter("xs", [NH, P, TS], BF16, isOutput=False)
    wg_d = nc.declare_dram_parameter("wg", [E_LOC, P, NH, I], BF16, isOutput=False)
    wu_d = nc.declare_dram_parameter("wu", [E_LOC, P, NH, I], BF16, isOutput=False)
    wd_d = nc.declare_dram_parameter("wd", [E_LOC, P, NI, H], F8, isOutput=False)
    bg_d = nc.declare_dram_parameter("bg", [P, E_LOC, NI], F32, isOutput=False)
    bu_d = nc.declare_dram_parameter("bu", [P, E_LOC, NI], F32, isOutput=False)
    wgus_d = nc.declare_dram_parameter("wgus", [P, NH, 2 * I], BF16, isOutput=False)
    wds_d = nc.declare_dram_parameter("wds", [P, NI, H], BF16, isOutput=False)
    bgus_d = nc.declare_dram_parameter("bgus", [P, 2 * NI], F32, isOutput=False)
    # combine weight per slot, tile-major: wcol[p, jt] = w of slot jt*128+p
    wcol_d = nc.declare_dram_parameter("wcol", [P, NTILE], F32, isOutput=False)
    yg_d = nc.declare_dram_parameter("yg", [NSLOT, H], BF16, isOutput=True)
    ys_d = nc.declare_dram_parameter("ys", [TS, H], BF16, isOutput=True)

    with tile.TileContext(nc) as tc:
        with (
            tc.tile_pool(name="wres", bufs=1) as wres,
            tc.tile_pool(name="xsb", bufs=1) as xsb,
            tc.tile_pool(name="xtb", bufs=2) as xtb,
            tc.tile_pool(name="hgep", bufs=2) as hgep,
            tc.tile_pool(name="hgsp", bufs=1) as hgsp,
            tc.tile_pool(name="actp", bufs=2) as actp,
            tc.tile_pool(name="outp", bufs=4) as outp,
            tc.tile_pool(name="ps_g", bufs=1, space="PSUM") as ps_g,
            tc.tile_pool(name="ps_u", bufs=1, space="PSUM") as ps_u,
            tc.tile_pool(name="ps_d", bufs=2, space="PSUM") as ps_d,
        ):
            # ---------- small constants (gpsimd = SWDGE ring, first) ---------
            bg_sb = wres.tile([P, E_LOC, NI], F32, tag="bg")
            nc.gpsimd.dma_start(bg_sb[:], bg_d[:])
            bu_sb = wres.tile([P, E_LOC, NI], F32, tag="bu")
            nc.gpsimd.dma_start(bu_sb[:], bu_d[:])
            wcol_sb = wres.tile([P, NTILE], F32, tag="wcol")
            nc.gpsimd.dma_start(wcol_sb[:], wcol_d[:])
            bgus_sb = wres.tile([P, 2 * NI], F32, tag="bgus")
            nc.gpsimd.dma_start(bgus_sb[:], bgus_d[:])
            # NOTE: no dma_start may ever be issued from the Scalar engine --
            # the silu activations queue behind them in its FIFO and DMA
            # issues are flow-controlled by transfer completions (measured:
            # first silu delayed to t=101us by 60 queued weight loads).
            # routed expert weights: wg on gpsimd (loads while sync stages x)
            wg_bf = {}
            wu_bf = {}
            wd_bf = {}
            for e in range(E_LOC):
                for h in range(NH):
                    t = wres.tile([P, I], BF16, tag=f"wg{e}_{h}", name="wg_h")
                    nc.gpsimd.dma_start(t[:], wg_d[e][:, h, :])
                    wg_bf[(e, h)] = t
            wds_sb = []
            for i in range(NI):
                t = wres.tile([P, H], BF16, tag=f"wds{i}", name="wds_i")
                nc.gpsimd.dma_start(t[:], wds_d[:, i, :])
                wds_sb.append(t)

            # ---------- gathered-x staging: one whole-capacity tile per h ----
            # (8 DMAs x 288KB per expert: big transfers, 2.3KB lines)
            def stage_expert(e):
                ts = {}
                for h in range(NH):
                    xt = xtb.tile([P, CAP], BF16, tag=f"xg{h}", name=f"xg{h}")
                    nc.sync.dma_start(xt[:], xg_d[h][:, e * CAP:(e + 1) * CAP])
                    ts[h] = xt
                return ts

            def load_wuwd(e):
                for h in range(NH):
                    t = wres.tile([P, I], BF16, tag=f"wu{e}_{h}", name="wu_h")
                    nc.sync.dma_start(t[:], wu_d[e][:, h, :])
                    wu_bf[(e, h)] = t
                t = wres.tile([P, NI, H], F8, tag=f"wd{e}", name="wd_e")
                nc.sync.dma_start(t[:], wd_d[e])
                wd_bf[e] = t

            # PE warmup: ~4.3us of dummy matmuls while the first DMAs land,
            # so HAM un-throttles to K=8/8 before the first real matmul
            warm = wres.tile([P, 512], BF16, tag="warm")
            nc.vector.memset(warm[:], 0.0)
            for _ in range(10):
                pw = ps_d.tile([P, 512], F32, tag="d", name="pwarm")
                nc.tensor.matmul(pw[:], warm[:, 0:P], warm[:],
                                 start=True, stop=True)

            # sync queue order = consumption order: e0 x + weights, e1 x +
            # weights, then the shared-expert tensors (shared runs LAST so the
            # slow first-100us DMA window is hidden under routed compute),
            # then the remaining expert weights
            staged = {0: stage_expert(0)}
            load_wuwd(0)
            staged[1] = stage_expert(1)
            load_wuwd(1)
            xs_sb = []
            wgus_sb = []
            for h in range(NH):
                t = xsb.tile([P, TS], BF16, tag=f"xs{h}")
                nc.sync.dma_start(t[:], xs_d[h])
                xs_sb.append(t)
                t = wres.tile([P, 2 * I], BF16, tag=f"wgus{h}", name="wgus_h")
                nc.sync.dma_start(t[:], wgus_d[:, h, :])
                wgus_sb.append(t)
            load_wuwd(2)
            load_wuwd(3)

            # ---------- routed experts over gathered slots -------------------
            # gate/up: one stationary load serves the 3 moving chunks; down:
            # one hge stationary serves both output halves' weight slices
            for e in range(E_LOC):
                if e + 2 < E_LOC:
                    staged[e + 2] = stage_expert(e + 2)
                xt = staged.pop(e)
                hge = {}
                deferred = {}

                def drain_gu(e, i, c, pg, pu):
                    ga = actp.tile([P, CN], F32, tag=f"gact{c}", name="ga")
                    nc.scalar.activation(ga[:], pg[:, 0:CN], AF.Silu,
                                         bias=bg_sb[:, e, i:i + 1])
                    # fp8 pair tile [P, 2, CN]: i-pair member in dim1, feeds
                    # the DoubleRow down matmul (K=256 per instruction)
                    k, m = divmod(i, 2)
                    if m == 0:
                        hge[(k, c)] = hgep.tile([P, 2, CN], F8,
                                                tag=f"hge{k}_{c}", name="ht")
                    nc.vector.scalar_tensor_tensor(
                        hge[(k, c)][:, m, :], pu[:, 0:CN], bu_sb[:, e, i:i + 1],
                        ga[:], ALU.add, ALU.mult)

                for i in range(NI):
                    pgs = [ps_g.tile([P, 512], F32, tag=f"g{c}", name="pg")
                           for c in range(NC)]
                    for h in range(NH):
                        for c in range(NC):
                            nc.tensor.matmul(pgs[c][:, 0:CN],
                                             wg_bf[(e, h)][:, i * P:(i + 1) * P],
                                             xt[h][:, c * CN:(c + 1) * CN],
                                             start=(h == 0),
                                             stop=(h == NH - 1))
                    pus = [ps_u.tile([P, 512], F32, tag=f"u{c}", name="pu")
                           for c in range(NC)]
                    for h in range(NH):
                        for c in range(NC):
                            nc.tensor.matmul(pus[c][:, 0:CN],
                                             wu_bf[(e, h)][:, i * P:(i + 1) * P],
                                             xt[h][:, c * CN:(c + 1) * CN],
                                             start=(h == 0),
                                             stop=(h == NH - 1))
                    for c in range(NC):
                        if i == NI - 1 and c > 0:
                            # defer the last iteration's chunk-1/2 drains into
                            # the down phase: the ACT/DVE backlog at down
                            # start otherwise stalls the PSUM rotation
                            deferred[c] = (pgs[c], pus[c])
                        else:
                            drain_gu(e, i, c, pgs[c], pus[c])
                for j in range(NT_E):
                    c, jc = divmod(j, CN // P)
                    if jc == 0 and c in deferred:
                        pg, pu = deferred.pop(c)
                        drain_gu(e, NI - 1, c, pg, pu)
                    jt = e * NT_E + j
                    out_sb = outp.tile([P, H], BF16, tag="out", name="yg_out")
                    for half in range(2):
                        h0 = half * (H // 2)
                        pd = ps_d.tile([P, 512], F32, tag="d", name="pd")
                        for k in range(2):
                            nc.tensor.matmul(
                                pd[:],
                                hge[(k, c)][:, :, jc * P:(jc + 1) * P],
                                wd_bf[e][:, 2 * k:2 * k + 2, h0:h0 + H // 2],
                                start=(k == 0), stop=(k == 1),
                                perf_mode=mybir.MatmulPerfMode.DoubleRow)
                        # combine-weight scale while draining PSUM; halves
                        # split between ACT (Copy w/ scale) and DVE so neither
                        # engine gates the down phase
                        if half == 0:
                            nc.scalar.activation(out_sb[:, h0:h0 + H // 2],
                                                 pd[:], AF.Copy,
                                                 scale=wcol_sb[:, jt:jt + 1])
                        else:
                            nc.vector.tensor_tensor(
                                out_sb[:, h0:h0 + H // 2], pd[:],
                                wcol_sb[:, jt:jt + 1].broadcast_to([P, H // 2]),
                                ALU.mult)
                    s0 = e * CAP + j * P
                    nc.gpsimd.dma_start(yg_d[s0:s0 + P, :], out_sb[:])

            # ---------- shared expert (tokens TS*core .. TS*(core+1)) --------
            TC = 512
            hs = []
            for i in range(NI):
                psg = ps_g.tile([P, TC], F32, tag=f"g{i % 3}", name="psg")
                for h in range(NH):
                    nc.tensor.matmul(psg[:], wgus_sb[h][:, i * P:(i + 1) * P],
                                     xs_sb[h][:], start=(h == 0),
                                     stop=(h == NH - 1))
                psu = ps_u.tile([P, TC], F32, tag=f"u{i % 3}", name="psu")
                for h in range(NH):
                    nc.tensor.matmul(psu[:], wgus_sb[h][:, I + i * P:I + (i + 1) * P],
                                     xs_sb[h][:], start=(h == 0),
                                     stop=(h == NH - 1))
                gs = actp.tile([P, TC], F32, tag="gact", name="gs")
                nc.scalar.activation(gs[:], psg[:], AF.Silu,
                                     bias=bgus_sb[:, i:i + 1])
                hsi = hgsp.tile([P, TC], BF16, tag=f"hs{i}")
                nc.vector.scalar_tensor_tensor(hsi[:], psu[:],
                                               bgus_sb[:, NI + i:NI + i + 1],
                                               gs[:], ALU.add, ALU.mult)
                hs.append(hsi)
            for j in range(TS // P):
                out_sb = outp.tile([P, H], BF16, tag="out", name="ys_out")
                for half in range(2):
                    h0 = half * (H // 2)
                    pd = ps_d.tile([P, 512], F32, tag="d", name="pds")
                    for i in range(NI):
                        nc.tensor.matmul(pd[:], hs[i][:, j * P:(j + 1) * P],
                                         wds_sb[i][:, h0:h0 + H // 2],
                                         start=(i == 0), stop=(i == NI - 1))
                    # split the PSUM drain between ACT and DVE
                    if half == 0:
                        nc.scalar.activation(out_sb[:, h0:h0 + H // 2], pd[:],
                                             AF.Copy)
                    else:
                        nc.vector.tensor_copy(out_sb[:, h0:h0 + H // 2], pd[:])
                nc.gpsimd.dma_start(ys_d[j * P:(j + 1) * P, :], out_sb[:])

    nc.finalize()
    return nc


def _route(inputs):
    """Host-side router: top-8 selection, per-expert token lists, slot map."""
    x = np.ascontiguousarray(
        np.asarray(inputs["hidden_states"], np.float32)).reshape(T, H)
    Wr = np.asarray(inputs["Wr"], np.float32)
    br = np.asarray(inputs["br"], np.float32)
    logits = x @ Wr + br
    aff = 1.0 / (1.0 + np.exp(-logits))
    idx = np.argsort(-aff, axis=1, kind="stable")[:, :TOPK]        # [T, K]
    topv = np.take_along_axis(aff, idx, axis=1)
    topw = (topv / (topv.sum(1, keepdims=True) + 1e-9)).astype(np.float32)
    w_full = np.zeros((T, E), np.float32)
    np.put_along_axis(w_full, idx, topw, axis=1)

    tok_ids = np.full((E, CAP), -1, np.int64)   # token id per slot (-1 = pad)
    w_slot = np.zeros((E, CAP), np.float32)     # combine weight per slot
    # global slot index for each (token, expert) pair; -1 if not routed/dropped
    pos = np.full((T, E), -1, np.int64)
    for e in range(E):
        tl = np.nonzero(w_full[:, e] > 0)[0]
        if len(tl) > CAP:   # overflow: drop the smallest-weight tokens
            keep = np.argsort(-w_full[tl, e], kind="stable")[:CAP]
            tl = np.sort(tl[keep])
        c = e // E_LOC
        el = e % E_LOC
        base = c * NSLOT + el * CAP
        tok_ids[e, :len(tl)] = tl
        w_slot[e, :len(tl)] = w_full[tl, e]
        pos[tl, e] = base + np.arange(len(tl))
    slot_of = np.take_along_axis(pos, idx, axis=1)                 # [T, K]
    if (slot_of < 0).any():
        # dropped pairs: point at any zero-weight (padded) slot of the owning
        # core -- guaranteed to exist (sum of local loads <= T < NSLOT) and its
        # device output is exactly 0 (combine weight 0)
        flat_w = w_slot.reshape(NCORES, NSLOT)
        own_core = idx // E_LOC
        for c in range(NCORES):
            z = int(np.nonzero(flat_w[c] == 0)[0][0]) + c * NSLOT
            slot_of[(slot_of < 0) & (own_core == c)] = z
    return x, w_full, tok_ids, w_slot, slot_of


def prep(inputs):
    """Host routing + sharding: returns (per-core input maps, aux for assembly)."""
    import ml_dtypes
    bf = ml_dtypes.bfloat16

    x, w_full, tok_ids, w_slot, slot_of = _route(inputs)
    Wg = np.asarray(inputs["Wg"], np.float32)
    bg = np.asarray(inputs["bg"], np.float32)
    Wu = np.asarray(inputs["Wu"], np.float32)
    bu = np.asarray(inputs["bu"], np.float32)
    Wd = np.asarray(inputs["Wd"], np.float32)
    bd = np.asarray(inputs["bd"], np.float32)
    Wg_s = np.asarray(inputs["Wg_s"], np.float32)
    bg_s = np.asarray(inputs["bg_s"], np.float32)
    Wu_s = np.asarray(inputs["Wu_s"], np.float32)
    bu_s = np.asarray(inputs["bu_s"], np.float32)
    Wd_s = np.asarray(inputs["Wd_s"], np.float32)
    bd_s = np.asarray(inputs["bd_s"], np.float32)

    f8 = ml_dtypes.float8_e4m3

    xT = np.ascontiguousarray(x.T.astype(bf))                      # [H, T]
    # Wu/bu are pre-scaled by SHS so hge = SHS*h fits fp8 e4m3 well; Wd is
    # quantized to fp8 with scale SWD. Both scales are folded into wcol on
    # the host (and divided out of ys after the run).
    wgus = np.concatenate([Wg_s, Wu_s], axis=1)                    # [H, 2I]
    wgus_c = np.ascontiguousarray(
        wgus.reshape(NH, P, 2 * I).transpose(1, 0, 2).astype(bf))
    wds_c = np.ascontiguousarray(
        Wd_s.reshape(NI, P, H).transpose(1, 0, 2).astype(bf))
    bgus_c = np.ascontiguousarray(
        np.stack([bg_s.reshape(NI, P), bu_s.reshape(NI, P)], 0)
        .reshape(2 * NI, P).T)
    # host-side bias term: sum_e w[t,e]*bd[e] plus the shared expert's bd_s
    bias_host = w_full @ bd + bd_s                                 # [T, H]

    in_maps = []
    for c in range(NCORES):
        loc = list(range(c * E_LOC, (c + 1) * E_LOC))
        cols = tok_ids[loc].reshape(-1).clip(0)                    # [NSLOT]
        xg = xT[:, cols].reshape(NH, P, NSLOT)
        wcol = np.ascontiguousarray(
            w_slot[loc].reshape(NTILE, P).T / (SWD * SHS))         # [P,NTILE]
        in_maps.append({
            "xg": np.ascontiguousarray(xg),
            "xs": np.ascontiguousarray(
                xT[:, c * TS:(c + 1) * TS].reshape(NH, P, TS)),
            "wg": np.ascontiguousarray(
                Wg[loc].reshape(E_LOC, NH, P, I).transpose(0, 2, 1, 3).astype(bf)),
            "wu": np.ascontiguousarray(
                (SHS * Wu[loc]).reshape(E_LOC, NH, P, I).transpose(0, 2, 1, 3).astype(bf)),
            "wd": np.ascontiguousarray(
                (SWD * Wd[loc]).reshape(E_LOC, NI, P, H).transpose(0, 2, 1, 3).astype(f8)),
            "bg": np.ascontiguousarray(bg[loc].reshape(E_LOC, NI, P).transpose(2, 0, 1)),
            "bu": np.ascontiguousarray(
                SHS * bu[loc].reshape(E_LOC, NI, P).transpose(2, 0, 1)),
            "wgus": wgus_c,
            "wds": wds_c,
            "bgus": bgus_c,
            "wcol": wcol,
        })
    return in_maps, (slot_of, bias_host)


def prep_inputs(inputs):
    return prep(inputs)[0]


def assemble_output(results, aux):
    """shared slices + weighted routed contributions + host-side bias term."""
    slot_of, bias_host = aux
    y = np.empty((T, H), np.float32)
    for c in range(NCORES):
        y[c * TS:(c + 1) * TS] = results[c]["ys"].astype(np.float32)
    down = np.concatenate([results[c]["yg"] for c in range(NCORES)], axis=0)
    y += down[slot_of].astype(np.float32).sum(axis=1)
    y += bias_host
    return y


_CACHE = {}


def get_runner():
    """Build + jit once; returns run(in_maps) -> list of per-core output dicts."""
    if "run" in _CACHE:
        return _CACHE["run"]
    import jax
    from jax.sharding import Mesh, PartitionSpec
    from jax.experimental.shard_map import shard_map
    from concourse import bass2jax

    nc = build_nc()
    bass2jax.install_neuronx_cc_hook()

    in_names = []
    out_names = []
    out_avals = []
    partition_name = nc.partition_id_tensor.name if nc.partition_id_tensor else None
    for alloc in nc.m.functions[0].allocations:
        if not isinstance(alloc, mybir.MemoryLocationSet):
            continue
        name = alloc.memorylocations[0].name
        if alloc.kind == "ExternalInput":
            if name != partition_name:
                in_names.append(name)
        elif alloc.kind == "ExternalOutput":
            out_names.append(name)
            out_avals.append(
                jax.core.ShapedArray(tuple(alloc.tensor_shape),
                                     mybir.dt.np(alloc.dtype)))
    n_params = len(in_names)
    n_outs = len(out_names)
    all_names = in_names + out_names + ([partition_name] if partition_name else [])
    donate = tuple(range(n_params, n_params + n_outs))

    def _body(*args):
        operands = list(args)
        if partition_name is not None:
            operands.append(bass2jax.partition_id_tensor())
        return tuple(bass2jax._bass_exec_p.bind(
            *operands,
            out_avals=tuple(out_avals),
            in_names=tuple(all_names),
            out_names=tuple(out_names),
            lowering_input_output_aliases=(),
            sim_require_finite=True,
            sim_require_nnan=True,
            nc=nc,
        ))

    devices = jax.devices()[:NCORES]
    mesh = Mesh(np.asarray(devices), ("core",))
    in_specs = (PartitionSpec("core"),) * (n_params + n_outs)
    out_specs = (PartitionSpec("core"),) * n_outs
    sharded = jax.jit(
        shard_map(_body, mesh=mesh, in_specs=in_specs, out_specs=out_specs,
                  check_rep=False),
        donate_argnums=donate, keep_unused=True)

    def run(in_maps, dev_inputs=None):
        if dev_inputs is None:
            dev_inputs = [
                np.concatenate([np.asarray(in_maps[c][n]) for c in range(NCORES)],
                               axis=0)
                for n in in_names
            ]
        zeros = [np.zeros((NCORES * a.shape[0], *a.shape[1:]), a.dtype)
                 for a in out_avals]
        outs = sharded(*dev_inputs, *zeros)
        return [
            {name: np.asarray(outs[i]).reshape(NCORES, *out_avals[i].shape)[c]
             for i, name in enumerate(out_names)}
            for c in range(NCORES)
        ]

    _CACHE["run"] = run
    _CACHE["meta"] = (in_names, out_names, out_avals, sharded, mesh)
    return run


def kernel(**inputs) -> np.ndarray:
    run = get_runner()
    in_maps, aux = prep(inputs)
    results = run(in_maps)
    return assemble_output(results, aux).reshape(B, S, H).astype(np.float32)
